# revision 38
# baseline (speedup 1.0000x reference)
"""Trainium2 Bass kernel for nn_KCLWONegLoss (raw bass, no TileContext).

Reference math (all f32):
    sums    = embs.sum(axis=1)                          # [64, 512]
    pos[p]  = cos(sums[p], sums[p+8])                   # p in 0..55
    a       = g1[neg1]; b = g2[neg2]                    # [56, 32, 512]
    sim[p,d]= cos over K axis (32) of a[p,:,d], b[p,:,d]
    num     = exp(pos/0.1)
    den     = num + sum_d exp(sim/0.1)
    loss    = 2 * sum_p (log(den) - pos/0.1)

Sharding: data-parallel over the D=64 group axis (8 groups/core) for the
embs reduction; the 56 positive pairs are sharded 7/core, each core
receiving only its gathered rows of g1/g2 (row-gather host-side). The
final 56 cosines + log-sum (~0.1 Mflop) run on host in float64.

Device schedule (per core, HBM-stream-bound at ~358 GB/s): all inputs
stream contiguously on the SP HWDGE ring in consumption order (gather
rows, then the 8 embs group chunks; group 7 split into row-halves that
feed matmuls directly so only one matmul + a [1,512] copy + a small DMA
sit after the last input byte). Each embs chunk [256,512] loads as
[128, 2, 512] (partition p = rows 2p, 2p+1); the 256->128 pre-reduction
is a DVE tensor_add of the halves, then one 8-col selector matmul per
group accumulates group sums in PSUM (chain stops at g6 so rows 0..6
copy out early). Negative path: a*b on GpSimd, squares on DVE's early
idle window, block-selector matmuls on PE, sim = dot*rsqrt(asq)*
rsqrt(bsq), Exp with accum_out writing the den column straight into the
out tile. Hand-managed semaphores replace the TileContext barriers:

  sem_d[i] : one per SP-ring transfer (+16 at completion, waited >=16).
             A shared cumulative sem would be racy: SDMA engines progress
             unevenly across queued transfers, so an intermediate
             threshold can be crossed by later transfers' per-engine
             increments while an earlier transfer is incomplete.
  sem_c    : consts DMA on the ACT ring (+16)
  sem_out  : the two output DMAs (+16 each, final wait at the exact
             total 32, which only all shares of both can reach)
  sem_dve / sem_gps / sem_pe / sem_act : per-engine op counters (+1)

A final all-engine barrier keeps the NEFF-wrapper epilogue (which resets
all semaphores) from racing the in-flight waits.
"""

import numpy as np

D, NG, DIM = 64, 256, 512
L, K = 8, 32
P = D - L
TEMP = 0.1
EPS = 1e-8
N_CORES = 8
GPC = D // N_CORES
PPC = P // N_CORES

_PROGRAM = None
LAST_RESULTS = None


def _build_program():
    from contextlib import ExitStack

    import concourse.bass as bass
    from concourse import bacc, mybir

    f32 = mybir.dt.float32
    f32r = mybir.dt.float32r
    AF = mybir.ActivationFunctionType
    nc = bacc.Bacc("TRN2", target_bir_lowering=False, debug=False)

    embs_t = nc.dram_tensor("embs_s", [GPC, NG, DIM], f32, kind="ExternalInput")
    gabA_t = nc.dram_tensor("gabA", [128, 2, DIM], f32, kind="ExternalInput")
    gabB_t = nc.dram_tensor("gabB", [96, 2, DIM], f32, kind="ExternalInput")
    consts_t = nc.dram_tensor("consts", [128, 81], f32, kind="ExternalInput")
    out_t = nc.dram_tensor("out", [PPC, DIM + 1], f32, kind="ExternalOutput")
    out7_t = nc.dram_tensor("out7", [1, DIM], f32, kind="ExternalOutput")

    ctx = ExitStack()
    with ctx:
        sb = lambda name, shape, dt: ctx.enter_context(
            nc.sbuf_tensor(name, shape, dt)
        ).ap()
        ps = lambda name, shape: ctx.enter_context(
            nc.psum_tensor(name, shape, f32)
        ).ap()
        sem = lambda name: ctx.enter_context(nc.semaphore(name))

        gab = sb("gab", [128, 4, DIM], f32)
        consts = sb("consts_sb", [128, 81], f32r)
        etiles = [sb(f"e{g}", [128, 2, DIM], f32r) for g in range(GPC - 2)]
        e6h0 = sb("e6h0", [128, DIM], f32r)
        e6h1 = sb("e6h1", [128, DIM], f32r)
        e7h0 = sb("e7h0", [128, DIM], f32r)
        e7h1 = sb("e7h1", [128, DIM], f32r)
        prods = [
            (sb(f"pr{t}", [128, DIM], f32r), sb(f"aa{t}", [128, DIM], f32r),
             sb(f"bb{t}", [128, DIM], f32r))
            for t in range(2)
        ]
        ctiles = [sb(f"c{g}", [128, DIM], f32r) for g in range(GPC - 2)]
        out_sb = sb("out_sb", [GPC, DIM + 1], f32)
        out7_sb = sb("out7_sb", [1, DIM], f32)
        dot_sb = sb("dot_sb", [8, DIM], f32)
        ai = sb("ai", [8, DIM], f32)
        bi = sb("bi", [8, DIM], f32)
        tmp = sb("tmp", [8, DIM], f32)
        sim = sb("sim", [8, DIM], f32)
        etile = sb("etile", [8, DIM], f32)

        dot_ps = ps("dot_ps", [8, DIM])
        asq_ps = ps("asq_ps", [8, DIM])
        bsq_ps = ps("bsq_ps", [8, DIM])
        sums_ps = ps("sums_ps", [8, DIM])
        s7_ps = ps("s7_ps", [1, DIM])

        # one sem per SP-ring transfer: a shared cumulative sem would be
        # racy (SDMA engines progress unevenly across queued transfers, so
        # an intermediate threshold can be crossed by later transfers'
        # per-engine increments while an earlier transfer is incomplete)
        sem_d = [sem(f"sem_d{i}") for i in range(12)]
        sem_c = sem("sem_c")
        sem_out = sem("sem_out")
        sem_dve = sem("sem_dve")
        sem_gps = sem("sem_gps")
        sem_pe = sem("sem_pe")
        sem_act = sem("sem_act")

        # ---- ACT ring: consts ----
        nc.scalar.dma_start(consts, consts_t.ap().bitcast(f32r)).then_inc(sem_c, 16)

        # ---- SP ring: gather + embs stream, FIFO completion order ----
        nc.sync.dma_start(gab[:, 0:2, :], gabA_t.ap()).then_inc(sem_d[0], 16)
        nc.sync.dma_start(gab[0:96, 2:4, :], gabB_t.ap()).then_inc(sem_d[1], 16)
        for g in range(GPC - 2):
            src = embs_t.ap()[g].rearrange("(p h) d -> p h d", h=2).bitcast(f32r)
            nc.sync.dma_start(etiles[g], src).then_inc(sem_d[2 + g], 16)
        src6 = embs_t.ap()[GPC - 2].rearrange("(p h) d -> p h d", h=2).bitcast(f32r)
        nc.sync.dma_start(e6h0, src6[:, 0, :]).then_inc(sem_d[8], 16)
        nc.sync.dma_start(e6h1, src6[:, 1, :]).then_inc(sem_d[9], 16)
        src7 = embs_t.ap()[GPC - 1].rearrange("(p h) d -> p h d", h=2).bitcast(f32r)
        nc.sync.dma_start(e7h0, src7[:, 0, :]).then_inc(sem_d[10], 16)
        nc.sync.dma_start(e7h1, src7[:, 1, :]).then_inc(sem_d[11], 16)

        # ---- GpSimd: pad memset, a*b products, tmp/sim ----
        nc.gpsimd.memset(gab[96:128, 2:4, :], 1.0).then_inc(sem_gps, 1)
        with nc.allow_low_precision(reason="f32r is fp32-width"):
            nc.gpsimd.wait_ge(sem_d[0], 16)
            nc.gpsimd.tensor_mul(prods[0][0], gab[:, 0, :], gab[:, 1, :]).then_inc(
                sem_gps, 1
            )
            nc.gpsimd.wait_ge(sem_d[1], 16)
            nc.gpsimd.tensor_mul(prods[1][0], gab[:, 2, :], gab[:, 3, :]).then_inc(
                sem_gps, 1
            )
            # tmp = dot * ai, sim = tmp * bi
            nc.gpsimd.wait_ge(sem_act, 2)
            nc.gpsimd.tensor_mul(tmp, dot_sb, ai).then_inc(sem_gps, 1)
            nc.gpsimd.wait_ge(sem_act, 3)
            nc.gpsimd.tensor_mul(sim, tmp, bi).then_inc(sem_gps, 1)

            # ---- DVE: squares then h-adds, stream-gated ----
            nc.vector.wait_ge(sem_d[0], 16)
            nc.vector.tensor_mul(prods[0][1], gab[:, 0, :], gab[:, 0, :]).then_inc(
                sem_dve, 1
            )
            nc.vector.tensor_mul(prods[0][2], gab[:, 1, :], gab[:, 1, :]).then_inc(
                sem_dve, 1
            )
            nc.vector.wait_ge(sem_d[1], 16)
            nc.vector.wait_ge(sem_gps, 1)   # pad memset
            nc.vector.tensor_mul(prods[1][1], gab[:, 2, :], gab[:, 2, :]).then_inc(
                sem_dve, 1
            )
            nc.vector.tensor_mul(prods[1][2], gab[:, 3, :], gab[:, 3, :]).then_inc(
                sem_dve, 1
            )
            for g in range(GPC - 2):
                nc.vector.wait_ge(sem_d[2 + g], 16)
                nc.vector.tensor_add(
                    ctiles[g], etiles[g][:, 0, :], etiles[g][:, 1, :]
                ).then_inc(sem_dve, 1)

        # group-7 copy on the (idle) DVE so it overlaps the out DMA issue
        nc.vector.wait_ge(sem_pe, 16)
        nc.vector.tensor_copy(out7_sb, s7_ps).then_inc(sem_dve, 1)

        # ---- PE: negative-path matmuls, selector matmuls, group 7 ----
        nc.tensor.wait_ge(sem_c, 16)
        nc.tensor.wait_ge(sem_gps, 2)
        nc.tensor.matmul(
            dot_ps, consts[:, 64:72], prods[0][0], start=True, stop=False
        ).then_inc(sem_pe, 1)
        nc.tensor.wait_ge(sem_dve, 2)
        nc.tensor.matmul(
            asq_ps, consts[:, 64:72], prods[0][1], start=True, stop=False
        ).then_inc(sem_pe, 1)
        nc.tensor.matmul(
            bsq_ps, consts[:, 64:72], prods[0][2], start=True, stop=False
        ).then_inc(sem_pe, 1)
        nc.tensor.wait_ge(sem_gps, 3)
        nc.tensor.matmul(
            dot_ps, consts[:, 72:80], prods[1][0], start=False, stop=True
        ).then_inc(sem_pe, 1)
        nc.tensor.wait_ge(sem_dve, 4)
        nc.tensor.matmul(
            asq_ps, consts[:, 72:80], prods[1][1], start=False, stop=True
        ).then_inc(sem_pe, 1)
        nc.tensor.matmul(
            bsq_ps, consts[:, 72:80], prods[1][2], start=False, stop=True
        ).then_inc(sem_pe, 1)
        for g in range(GPC - 2):
            nc.tensor.wait_ge(sem_dve, 5 + g)
            nc.tensor.matmul(
                sums_ps,
                consts[:, 8 * g:8 * g + 8],
                ctiles[g],
                start=(g == 0),
                stop=False,
            ).then_inc(sem_pe, 1)
        g6sel = consts[:, 8 * (GPC - 2):8 * (GPC - 2) + 8]
        nc.tensor.wait_ge(sem_d[8], 16)
        nc.tensor.matmul(
            sums_ps, g6sel, e6h0, start=False, stop=False
        ).then_inc(sem_pe, 1)
        nc.tensor.wait_ge(sem_d[9], 16)
        nc.tensor.matmul(
            sums_ps, g6sel, e6h1, start=False, stop=True
        ).then_inc(sem_pe, 1)
        nc.tensor.wait_ge(sem_d[10], 16)
        nc.tensor.matmul(
            s7_ps, consts[:, 80:81], e7h0, start=True, stop=False
        ).then_inc(sem_pe, 1)
        nc.tensor.wait_ge(sem_d[11], 16)
        nc.tensor.matmul(
            s7_ps, consts[:, 80:81], e7h1, start=False, stop=True
        ).then_inc(sem_pe, 1)

        # ---- ACT: dot copy, rsqrt's, exp(+den accum), output copies ----
        nc.scalar.wait_ge(sem_pe, 4)
        nc.scalar.copy(dot_sb, dot_ps).then_inc(sem_act, 1)
        nc.scalar.wait_ge(sem_pe, 5)
        nc.scalar.activation(ai, asq_ps, AF.Abs_reciprocal_sqrt).then_inc(sem_act, 1)
        nc.scalar.wait_ge(sem_pe, 6)
        nc.scalar.activation(bi, bsq_ps, AF.Abs_reciprocal_sqrt).then_inc(sem_act, 1)
        nc.scalar.wait_ge(sem_gps, 5)
        nc.scalar.activation(
            etile, sim, AF.Exp,
            scale=float(1.0 / TEMP), accum_out=out_sb[:, DIM:DIM + 1],
        ).then_inc(sem_act, 1)
        nc.scalar.wait_ge(sem_pe, 14)
        nc.scalar.copy(out_sb[0:PPC, 0:DIM], sums_ps[0:PPC, :]).then_inc(sem_act, 1)

        # ---- outputs on the ACT ring: its completion-receipt pipeline is
        # idle (the SP ring is still draining ~5MB of input receipts, which
        # previously exposed ~3.5us of output-receipt wait on the tail) ----
        nc.scalar.dma_start(out_t.ap(), out_sb[0:PPC, :]).then_inc(sem_out, 16)
        nc.scalar.wait_ge(sem_dve, 11)      # DVE copy of the group-7 row
        nc.scalar.dma_start(out7_t.ap(), out7_sb).then_inc(sem_out, 16)
        nc.sync.wait_ge(sem_out, 32)

        # keep the wrapper epilogue (sem resets) from racing our waits;
        # sem-only: engines execute in order, so reaching the barrier
        # already implies all prior compute retired
        nc.all_engine_barrier(sem_only=True)

        nc.compile()
    return nc


def _get_program():
    global _PROGRAM
    if _PROGRAM is None:
        _PROGRAM = _build_program()
    return _PROGRAM


def kernel(embs, g0, g1, g2, neg1, neg2, **_unused):
    global LAST_RESULTS
    from concourse.bass_utils import run_bass_kernel_spmd

    embs = np.ascontiguousarray(np.asarray(embs, dtype=np.float32))
    g1 = np.ascontiguousarray(np.asarray(g1, dtype=np.float32))
    g2 = np.ascontiguousarray(np.asarray(g2, dtype=np.float32))
    neg1 = np.asarray(neg1).astype(np.int64)
    neg2 = np.asarray(neg2).astype(np.int64)

    consts = np.zeros((128, 81), np.float32)
    for g in range(GPC):
        consts[:, 8 * g + g] = 1.0
    for m in range(4):
        consts[m * 32:(m + 1) * 32, 64 + m] = 1.0
    for j in range(3):
        consts[j * 32:(j + 1) * 32, 72 + 4 + j] = 1.0
    consts[96:128, 79] = 1.0
    consts[:, 80] = 1.0

    in_maps = []
    for c in range(N_CORES):
        idx1 = neg1[c * PPC:(c + 1) * PPC].reshape(-1)
        idx2 = neg2[c * PPC:(c + 1) * PPC].reshape(-1)
        gabA = np.empty((128, 2, DIM), np.float32)
        gabA[:, 0, :] = g1[idx1[:128]]
        gabA[:, 1, :] = g2[idx2[:128]]
        gabB = np.empty((96, 2, DIM), np.float32)
        gabB[:, 0, :] = g1[idx1[128:]]
        gabB[:, 1, :] = g2[idx2[128:]]
        in_maps.append({
            "embs_s": embs[c * GPC:(c + 1) * GPC],
            "gabA": gabA,
            "gabB": gabB,
            "consts": consts,
        })

    nc = _get_program()
    res = run_bass_kernel_spmd(nc, in_maps, core_ids=list(range(N_CORES)))
    LAST_RESULTS = res

    sums = np.empty((D, DIM), np.float64)
    den_neg = np.empty((P,), np.float64)
    for c in range(N_CORES):
        o = res.results[c]["out"]
        sums[c * GPC:c * GPC + PPC] = o[:, :DIM]
        sums[c * GPC + GPC - 1] = res.results[c]["out7"][0]
        den_neg[c * PPC:(c + 1) * PPC] = o[:, DIM]

    s_i, s_j = sums[:P], sums[L:]
    na = np.maximum(np.sqrt((s_i * s_i).sum(1)), EPS)
    nb = np.maximum(np.sqrt((s_j * s_j).sum(1)), EPS)
    pos = (s_i * s_j).sum(1) / (na * nb)
    num = np.exp(pos / TEMP)
    den = num + den_neg
    total = 2.0 * np.sum(np.log(den) - pos / TEMP)
    return np.asarray(total, dtype=np.float32)
